# revision 9
# baseline (speedup 1.0000x reference)
"""Trainium2 Bass kernel for CDAttnBlock.

Reference computation (per batch element b, all in fp32):
    q,k,v   = split(x  @ Wqkv)   heads=12, d=64
    q2,k2,v2= split(x2 @ Wqkv)
    o1 = attn(q, k,  v);  o2 = attn(q2, k2, v2);  o3 = attn(q, k2, v2)
    y_i = merge(o_i) @ Wout + bout

Sharding: pure data-parallel over batch (B=8) across 8 NeuronCores.
Each core runs the identical program on its own batch element; no
collectives.

Layout strategy (per core):
  - x.T in SBUF [768, 1024] built via PE transposes (contraction for all
    projections is over hidden, which must sit on partitions).
  - q.T, k.T stored per head-pair as [128, 1024] (d on partitions), v
    stored natural per s-tile as [128, 12*65] with a ones column per
    head appended (65th col) so the attention a@v matmul also produces
    the softmax denominator row.
  - scores computed transposed: sT[s_k, s_q] = kT.T @ qT, softmax as
    exp (no max subtraction: scores are N(0,~1), fp32 exp is safe),
    denominator via the ones column, normalization by a K=1 broadcast
    matmul + DVE multiply.
  - o accumulated transposed [768, 1024]; output projection uses oT as
    the stationary operand so y comes out natural [1024, 768].
  - Matmuls run as float32r (full PE rate). fp32r is a distinct bit
    layout, so every matmul operand tile is produced by a compute op
    (DVE/ACT) with float32r output dtype; DMA-loaded weights are staged
    fp32 and cast on ScalarE.
"""

import numpy as np

import concourse.bass as bass
import concourse.tile as tile
from concourse import bacc, mybir
from concourse.bass_utils import run_bass_kernel_spmd
from concourse.masks import make_identity

F32 = mybir.dt.float32
F32R = mybir.dt.float32r
AF = mybir.ActivationFunctionType

HIDDEN = 768
HEADS = 12
D = 64
S = 1024
B = 8
SCALE = D ** -0.5
NPAIR = HEADS // 2          # 6 head pairs
KT = HIDDEN // 128          # 6 k-tiles over hidden
ST = S // 128               # 8 s-tiles
SQB = S // 512              # 2 s_q blocks of 512
VW = D + 1                  # 65: v columns + ones column


def _build_xt(nc, tc, x_ap, xT, ident):
    """DMA x natural and PE-transpose into f32r xT tiles [128, 1024] x 6."""
    xnat = tc.alloc_tile_pool(name="xnat", bufs=3, side="right")
    tpp = tc.alloc_tile_pool(name="tpp", bufs=2, space="PSUM")
    for st in range(ST):
        xn = xnat.tile([128, HIDDEN], F32, name="xn", tag="xn")
        nc.sync.dma_start(xn[:], x_ap[st * 128:(st + 1) * 128, :])
        for ht in range(KT):
            tp = tpp.tile([128, 128], F32, name="tp", tag="tp")
            nc.tensor.transpose(tp[:], xn[:, ht * 128:(ht + 1) * 128],
                                ident[:])
            nc.vector.tensor_copy(xT[ht][:, st * 128:(st + 1) * 128], tp[:])
    tpp.release()
    xnat.release()


def _qkv(nc, tc, w_dram, xT, qT, kT, v_st, onescol):
    """Project xT -> qT/kT per head-pair and v natural per s-tile."""
    # --- v for all heads: lhsT = xT tile, rhs = Wv slice [128, 768] ---
    wvp = tc.alloc_tile_pool(name="wvp", bufs=1, side="right")
    wvstage = tc.alloc_tile_pool(name="wvstage", bufs=2, side="right")
    wv = []
    for kt in range(KT):
        f = wvstage.tile([128, HIDDEN], F32, name="wvf", tag="wvf")
        nc.sync.dma_start(
            f[:], w_dram[kt * 128:(kt + 1) * 128, 2 * HIDDEN:3 * HIDDEN])
        t = wvp.tile([128, HIDDEN], F32R, name=f"wv{kt}", tag=f"wv{kt}")
        nc.scalar.copy(t[:], f[:])
        wv.append(t)
    vps = tc.alloc_tile_pool(name="vps", bufs=2, space="PSUM")
    for st in range(ST):
        vp = vps.tile([128, HIDDEN], F32, name="vp", tag="vp")
        for kt in range(KT):
            xts = xT[kt][:, st * 128:(st + 1) * 128]
            nc.tensor.matmul(vp[:, 0:512], xts, wv[kt][:, 0:512],
                             start=(kt == 0), stop=(kt == KT - 1))
            nc.tensor.matmul(vp[:, 512:768], xts, wv[kt][:, 512:768],
                             start=(kt == 0), stop=(kt == KT - 1))
        # scatter [128, 12, 64] -> v_st[:, h, 0:64]; ones col 65th
        vs = v_st[st]
        nc.vector.tensor_copy(
            vs.rearrange("p (h w) -> p h w", w=VW)[:, :, 0:D],
            vp.rearrange("p (h w) -> p h w", w=D))
        nc.vector.tensor_copy(
            vs.rearrange("p (h w) -> p h w", w=VW)[:, :, D:VW],
            onescol[:, None, :].broadcast_to([128, HEADS, 1]))
    vps.release()
    wvstage.release()
    wvp.release()

    # --- qT / kT per head pair: lhsT = Wq/Wk col slice, rhs = xT ---
    wsl = tc.alloc_tile_pool(name="wsl", bufs=2, side="right")
    wslstage = tc.alloc_tile_pool(name="wslstage", bufs=3, side="right")
    qkps = tc.alloc_tile_pool(name="qkps", bufs=2, space="PSUM")
    for p in range(NPAIR):
        for which, base, dst in ((0, 0, qT), (1, HIDDEN, kT)):
            ws = []
            for kt in range(KT):
                f = wslstage.tile([128, 128], F32, name="wslf", tag="wslf")
                nc.sync.dma_start(
                    f[:], w_dram[kt * 128:(kt + 1) * 128,
                                 base + p * 128:base + (p + 1) * 128])
                t = wsl.tile([128, 128], F32R, name=f"wsl{which}{kt}",
                             tag=f"wsl{which}{kt}")
                nc.scalar.copy(t[:], f[:])
                ws.append(t)
            pp = qkps.tile([128, S], F32, name="qkp", tag="qkp")
            for kt in range(KT):
                for nb in range(2):
                    nc.tensor.matmul(
                        pp[:, nb * 512:(nb + 1) * 512], ws[kt][:],
                        xT[kt][:, nb * 512:(nb + 1) * 512],
                        start=(kt == 0), stop=(kt == KT - 1))
            nc.vector.tensor_copy(dst[p][:], pp[:])
    qkps.release()
    wslstage.release()
    wsl.release()


def _attn(nc, tc, qT, kT, v_st, oT, ones64, zbias):
    """oT[pair][64*hh:, :] = attention(q_h, k_h, v_h).T for both heads."""
    sps = tc.alloc_tile_pool(name="sps", bufs=2, space="PSUM")
    ovps = tc.alloc_tile_pool(name="ovps", bufs=2, space="PSUM")
    bcps = tc.alloc_tile_pool(name="bcps", bufs=2, space="PSUM")
    exps = tc.alloc_tile_pool(name="exps", bufs=3, side="right")
    smalls = tc.alloc_tile_pool(name="smalls", bufs=2, side="right")
    for p in range(NPAIR):
        for sqb in range(SQB):
            sq = slice(sqb * 512, (sqb + 1) * 512)
            for hh in range(2):
                hp = slice(hh * D, (hh + 1) * D)
                h = 2 * p + hh
                ov = ovps.tile([VW, 512], F32, name="ov", tag="ov")
                for kt in range(ST):
                    sp = sps.tile([128, 512], F32, name="sp", tag="sp")
                    nc.tensor.matmul(
                        sp[:], kT[p][hp, kt * 128:(kt + 1) * 128],
                        qT[p][hp, sq], start=True, stop=True)
                    ex = exps.tile([128, 512], F32R, name="ex", tag="ex")
                    nc.scalar.activation(ex[:], sp[:], AF.Exp,
                                         bias=zbias[:], scale=SCALE)
                    nc.tensor.matmul(
                        ov[:], v_st[kt].rearrange(
                            "q (h w) -> q h w", w=VW)[:, h, :],
                        ex[:], start=(kt == 0), stop=(kt == ST - 1))
                recf = smalls.tile([1, 512], F32, name="recf", tag="recf")
                nc.vector.reciprocal(recf[:], ov[D:VW, :])
                rec = smalls.tile([1, 512], F32R, name="rec", tag="rec")
                nc.vector.tensor_copy(rec[:], recf[:])
                bc = bcps.tile([D, 512], F32, name="bc", tag="bc")
                nc.tensor.matmul(bc[:], ones64[:], rec[:],
                                 start=True, stop=True)
                bcs = smalls.tile([D, 512], F32, name="bcs", tag="bcs")
                nc.vector.tensor_copy(bcs[:], bc[:])
                nc.vector.tensor_mul(oT[p][hp, sq], ov[0:D, :], bcs[:])
    smalls.release()
    exps.release()
    bcps.release()
    ovps.release()
    sps.release()


def _proj(nc, tc, oT, wout, bias_sb, y_dram):
    """y = oT.T @ Wout + bias, natural layout, DMA to DRAM."""
    yps = tc.alloc_tile_pool(name="yps", bufs=2, space="PSUM")
    ysb = tc.alloc_tile_pool(name="ysb", bufs=2, side="right")
    for st in range(ST):
        yp = yps.tile([128, HIDDEN], F32, name="yp", tag="yp")
        for ct in range(KT):
            ots = oT[ct][:, st * 128:(st + 1) * 128]
            nc.tensor.matmul(yp[:, 0:512], ots, wout[ct][:, 0:512],
                             start=(ct == 0), stop=(ct == KT - 1))
            nc.tensor.matmul(yp[:, 512:768], ots, wout[ct][:, 512:768],
                             start=(ct == 0), stop=(ct == KT - 1))
        yt = ysb.tile([128, HIDDEN], F32, name="yt", tag="yt")
        nc.vector.tensor_add(yt[:], yp[:], bias_sb[:])
        nc.sync.dma_start(y_dram[st * 128:(st + 1) * 128, :], yt[:])
    ysb.release()
    yps.release()


def build_kernel(ctx, tc, x, x2, wq, wo, bo, y1, y2, y3):
    nc = tc.nc

    const = ctx.enter_context(tc.tile_pool(name="const", bufs=1))
    ident = const.tile([128, 128], F32, name="ident")
    make_identity(nc, ident)
    ones64f = const.tile([1, D], F32, name="ones64f")
    nc.vector.memset(ones64f[:], 1.0)
    ones64 = const.tile([1, D], F32R, name="ones64")
    nc.vector.tensor_copy(ones64[:], ones64f[:])
    zbias = const.tile([128, 1], F32, name="zbias")
    nc.vector.memset(zbias[:], 0.0)
    onescol = const.tile([128, 1], F32, name="onescol")
    nc.vector.memset(onescol[:], 1.0)
    bias_sb = const.tile([128, HIDDEN], F32, name="bias_sb")
    bo_bcast = bass.AP(tensor=bo.tensor, offset=bo.offset,
                       ap=[[0, 128]] + list(bo.ap))
    nc.sync.dma_start(bias_sb[:], bo_bcast)

    woutp = ctx.enter_context(tc.tile_pool(name="woutp", bufs=1))
    wout = []
    for ct in range(KT):
        f = woutp.tile([128, HIDDEN], F32, name=f"woutf{ct}",
                       tag="woutf", bufs=2)
        nc.sync.dma_start(f[:], wo[ct * 128:(ct + 1) * 128, :])
        t = woutp.tile([128, HIDDEN], F32R, name=f"wout{ct}", tag=f"wout{ct}")
        nc.scalar.copy(t[:], f[:])
        wout.append(t)

    def persist(pool, shape, base, n, dtype=F32R):
        return [pool.tile(shape, dtype, name=f"{base}{i}", tag=f"{base}{i}")
                for i in range(n)]

    # ---- persistent q for x (lives until o3) ----
    qxp = ctx.enter_context(tc.tile_pool(name="qxp", bufs=1))
    qT_x = persist(qxp, [128, S], "qTx", NPAIR)

    # ================= phase A: qkv for x =================
    kvxp = tc.alloc_tile_pool(name="kvxp", bufs=1)
    kT_x = persist(kvxp, [128, S], "kTx", NPAIR)
    v_x = persist(kvxp, [128, HEADS * VW], "vx", ST)

    xtp = tc.alloc_tile_pool(name="xtp", bufs=1)
    xT = persist(xtp, [128, S], "xT", KT)
    _build_xt(nc, tc, x, xT, ident)
    _qkv(nc, tc, wq, xT, qT_x, kT_x, v_x, onescol)
    xtp.release()

    # ================= phase B: o1 = attn(q, k, v); y1 =================
    o1p = tc.alloc_tile_pool(name="o1p", bufs=1)
    oT1 = persist(o1p, [128, S], "oT1", NPAIR)
    _attn(nc, tc, qT_x, kT_x, v_x, oT1, ones64, zbias)
    _proj(nc, tc, oT1, wout, bias_sb, y1)
    o1p.release()
    kvxp.release()

    # ================= phase C: qkv for x2 =================
    kvx2p = tc.alloc_tile_pool(name="kvx2p", bufs=1)
    qT_x2 = persist(kvx2p, [128, S], "qTx2", NPAIR)
    kT_x2 = persist(kvx2p, [128, S], "kTx2", NPAIR)
    v_x2 = persist(kvx2p, [128, HEADS * VW], "vx2", ST)

    x2tp = tc.alloc_tile_pool(name="x2tp", bufs=1)
    x2T = persist(x2tp, [128, S], "x2T", KT)
    _build_xt(nc, tc, x2, x2T, ident)
    _qkv(nc, tc, wq, x2T, qT_x2, kT_x2, v_x2, onescol)
    x2tp.release()

    # ================= phase D: o2, y2, o3, y3 =================
    o2p = tc.alloc_tile_pool(name="o2p", bufs=1)
    oT2 = persist(o2p, [128, S], "oT2", NPAIR)
    _attn(nc, tc, qT_x2, kT_x2, v_x2, oT2, ones64, zbias)
    _proj(nc, tc, oT2, wout, bias_sb, y2)
    o2p.release()

    o3p = tc.alloc_tile_pool(name="o3p", bufs=1)
    oT3 = persist(o3p, [128, S], "oT3", NPAIR)
    _attn(nc, tc, qT_x, kT_x2, v_x2, oT3, ones64, zbias)
    _proj(nc, tc, oT3, wout, bias_sb, y3)
    o3p.release()
    kvx2p.release()


def build_bass():
    from contextlib import ExitStack
    nc = bacc.Bacc("TRN2", target_bir_lowering=False, debug=False,
                   num_devices=B)
    x = nc.dram_tensor("x", [S, HIDDEN], F32, kind="ExternalInput").ap()
    x2 = nc.dram_tensor("x2", [S, HIDDEN], F32, kind="ExternalInput").ap()
    wq = nc.dram_tensor("Wqkv", [HIDDEN, 3 * HIDDEN], F32,
                        kind="ExternalInput").ap()
    wo = nc.dram_tensor("Wout", [HIDDEN, HIDDEN], F32,
                        kind="ExternalInput").ap()
    bo = nc.dram_tensor("bout", [HIDDEN], F32, kind="ExternalInput").ap()
    y1 = nc.dram_tensor("y1", [S, HIDDEN], F32, kind="ExternalOutput").ap()
    y2 = nc.dram_tensor("y2", [S, HIDDEN], F32, kind="ExternalOutput").ap()
    y3 = nc.dram_tensor("y3", [S, HIDDEN], F32, kind="ExternalOutput").ap()
    with tile.TileContext(nc) as tc:
        with ExitStack() as ctx:
            build_kernel(ctx, tc, x, x2, wq, wo, bo, y1, y2, y3)
    nc.compile()
    return nc


def kernel(x, x2, Wqkv, Wout, bout):
    nc = build_bass()
    in_maps = [
        {"x": np.ascontiguousarray(x[b]), "x2": np.ascontiguousarray(x2[b]),
         "Wqkv": Wqkv, "Wout": Wout, "bout": bout}
        for b in range(B)
    ]
    res = run_bass_kernel_spmd(nc, in_maps, list(range(B)))
    y1 = np.stack([res.results[b]["y1"] for b in range(B)])
    y2 = np.stack([res.results[b]["y2"] for b in range(B)])
    y3 = np.stack([res.results[b]["y3"] for b in range(B)])
    return (y1, y2, y3)
